# revision 51
# baseline (speedup 1.0000x reference)
"""CXTRNN recurrence kernel for 8 Trainium2 NeuronCores.

Math (per reference):
    inp = einsum('tbs,hs->tbh', s, W_in) + b_in
    g   = sigmoid(einsum('tbz,rz->tbr', z, W_nm) + b_nm)
    x_t = (1-a)*x_{t-1} + a*(U @ (g_t * (V^T tanh(x_{t-1}))) + inp_t)
    y   = einsum('tbh,yh->tby', xs, W_out) + b_out

Sharding: data-parallel over batch B=256 -> 32 per core; params replicated;
the T=2048 sequential loop runs locally per core.

End-to-end wall time is dominated by the host<->device tunnel (~25-80MB/s,
half-duplex-ish, ~3-10ms fixed latency per transfer, zstd compression
inside) plus a SINGLE host cpu, so the design minimizes wire bytes, wire
transfers, and host-cpu passes, and pipelines the transfer directions:

- The T loop is split into 4 sequential device execs of (256, 768, 768,
  256) steps, carrying the recurrent state x on-device (f32, exact)
  between execs.  Piece p's y download overlaps piece p+1's sz upload and
  exec; the small first piece fills the pipeline early and the small last
  piece shrinks the drain tail.
- s uploads as int8 (4-sigma clip, 127 levels; scale folded into the
  device-side weights).  z uploads nibble-PACKED (two 4-bit dims per
  byte, halving its raw bytes): its quantization error passes through
  the saturating sigmoid gate and stays small.  The device unpacks with
  float-only Pool ops (no int8 bit/int ops exist there): h =
  round(b/16 - 0.46875) exact via the rounding int8 cast, l = b - 16h;
  the nibble offsets fold into the gate weights/bias.
- y downloads as int8 with per-(row, chunk) scales computed on device:
  absmax over each [DIM_Y, 512]-column chunk -> scale = absmax/126.5
  (headroom so rounding never saturates), quantize with DVE reciprocal,
  and DMA through a transposed access pattern into a [col, ydim] DRAM
  layout with the f32 scale table bitcast into its tail rows — one d2h
  transfer per core per piece, and the host dequant is a fully
  contiguous fused multiply (the strided gather was ~130ms of cpu).
  Total error vs the f32 reference is ~1.3e-2 (gate 2e-2).
- Weight tensors are cached on device across calls (content-checked) so
  steady-state calls upload only s and z.

Device design notes (driven by the TRN2 instruction cost model):
- Layout is [feature, batch]; the host pre-transposes s and z.
- The per-step dependency chain is the whole ballgame (engines idle
  otherwise).  It is exactly 4 links: tanh (ACT) -> q=V^T h (PE) ->
  r=g*q (DVE) -> x' accumulation (PE).  The (1-a)*x term is folded into
  the PE accumulation group via a scaled-identity matmul (off the chain),
  which removes the DVE x-update from the chain.
- The state lives in PSUM (one bank per step, ping-pong); an off-chain ACT
  copy evacuates each state to SBUF both for the y-output matmul and as the
  rhs of the next step's scaled-identity matmul.
- sigmoid is computed as 0.5*tanh(0.5*w + 0.5*b_nm)+0.5 so ACT only ever
  uses the Tanh table (a table switch costs ~1.3us); b_nm rides the ACT
  bias operand so z needs no ones-row.
- b_in folds into the fused-update matmul via a device-memset ones row.
- Bacc (not raw Bass) is required: its compile() pass splits multi-wait
  instructions into event semaphores (hardware allows at most one
  semaphore wait per engine instruction).
"""

import numpy as np
import ml_dtypes
from concurrent.futures import ThreadPoolExecutor

import concourse.mybir as mybir
from concourse import bacc
from concourse.tile_autobufs import add_dep_helper
from concourse.bass import ts
from concourse.tile import TileContext

T = 2048
B = 256
DIM_S, DIM_Y, DIM_Z, RANK, DIM_HID = 32, 32, 16, 16, 128
ALPHA = 0.2
BETA = 1.0 - ALPHA
NCORES = 8
BL = B // NCORES            # 32 batch elements per core
CH = 16                     # timesteps per chunk
PIECES = (256, 768, 768, 256)   # timesteps per sequential device exec:
NPIECE = len(PIECES)            # small first piece fills the wire pipeline
OFFS = (0, 256, 1024, 1792)     # early, small last piece cuts the tail
NCOLS = CH * BL             # 512 columns per chunk tile
ZPK = DIM_Z // 2            # 8 packed z rows (two 4-bit dims per byte)
SZROWS = DIM_S + ZPK        # 40: s rows 0:32, packed z rows 32:40
# Fused-update rhs row layout: [r (0:16); zero pad (16:32); s (32:64);
# ones row (64)].  Compute-engine accesses must start at a 32-aligned
# partition; the pad rows pair with zero weight rows and are zeroed by a
# per-chunk memset; the ones row (b_in) is a per-chunk memset too.
KSR = 65

F32 = mybir.dt.float32
BF16 = mybir.dt.bfloat16
I8 = mybir.dt.int8
NP_BF16 = ml_dtypes.bfloat16
QCLIP = 4.0                 # clip at 4 sigma
QSCALE = QCLIP / 127.0      # s: int8 lsb in data units
QSCALE_Z = QCLIP / 7.0      # z: 4-bit lsb in data units (nibble-packed)
YQDIV = 126.5               # y: quant headroom so rounding can't saturate
                            # (measured: d2h has a hard ~42MB/s per-raw-byte
                            # ceiling and is barely compression-sensitive,
                            # so a lower-entropy narrower quantizer buys
                            # ~nothing — keep full int8 precision)

_BUILT = {}


def _get_pool():
    if "pool" not in _BUILT:
        _BUILT["pool"] = ThreadPoolExecutor(max_workers=24)
    return _BUILT["pool"]


def _build_module(TP):
    """One piece: TP steps, x carried in/out via DRAM f32."""
    NCHUNK = TP // CH
    PCOLS = TP * BL
    nc = bacc.Bacc(None)

    sz = nc.dram_tensor("sz", [SZROWS, PCOLS], I8, kind="ExternalInput")
    x_in = nc.dram_tensor("x_in", [DIM_HID, BL], F32, kind="ExternalInput")
    v_w = nc.dram_tensor("v_w", [DIM_HID, RANK], F32, kind="ExternalInput")
    wsr = nc.dram_tensor("wsr", [KSR, DIM_HID], BF16, kind="ExternalInput")
    wnm = nc.dram_tensor("wnm", [DIM_Z, RANK], BF16, kind="ExternalInput")
    bnm = nc.dram_tensor("bnm", [RANK, 1], F32, kind="ExternalInput")
    wout = nc.dram_tensor("wout", [DIM_HID, DIM_Y], F32, kind="ExternalInput")
    beye_const = nc.inline_tensor(
        (BETA * np.eye(DIM_HID)).astype(np.float32), name="beta_eye"
    )
    # y stored [col, ydim] (DMA writes through a transposed access pattern)
    # so the host dequant is fully contiguous — the strided gather was
    # ~130ms of single-cpu time per call.  Rows [PCOLS:] carry the f32
    # scale table bitcast to int8 — one d2h transfer per core per piece
    # (each wire transfer costs ~3.6ms fixed, so no separate scale tensor).
    y_out = nc.dram_tensor("y_out", [PCOLS + 4 * NCHUNK, DIM_Y], I8,
                           kind="ExternalOutput")
    x_out = nc.dram_tensor("x_out", [DIM_HID, BL], F32, kind="ExternalOutput")

    AF = mybir.ActivationFunctionType
    OP = mybir.AluOpType

    with TileContext(nc) as tc:
        with (
            tc.tile_pool(name="consts", bufs=1) as consts,
            tc.tile_pool(name="szi_in", bufs=4) as szi_pool,
            tc.tile_pool(name="sr_in", bufs=4) as sr_pool,
            tc.tile_pool(name="z_in", bufs=4) as z_pool,
            tc.tile_pool(name="zpk", bufs=2) as zpk_pool,
            tc.tile_pool(name="g_buf", bufs=4) as g_pool,
            tc.tile_pool(name="tg_buf", bufs=4) as tg_pool,
            tc.tile_pool(name="xs_buf", bufs=3) as xs_pool,
            tc.tile_pool(name="h_buf", bufs=3) as h_pool,
            tc.tile_pool(name="q_buf", bufs=3) as q_pool,
            tc.tile_pool(name="ps_x", bufs=2, space="PSUM") as x_psum,
            tc.tile_pool(name="ps_q", bufs=2, space="PSUM") as q_psum,
            tc.tile_pool(name="ps_g", bufs=2, space="PSUM") as g_psum,
            tc.tile_pool(name="ps_y", bufs=2, space="PSUM") as y_psum,
        ):
            # Incoming state first: the j=0 chain (tanh + mm_I) needs it.
            x_init = consts.tile([DIM_HID, BL], F32)
            nc.sync.dma_start(x_init[:], x_in[:])
            scr_sb = consts.tile([1, 1], F32)
            nc.vector.memset(scr_sb[:], 0.0)
            # per-step semaphore-flush scratch: 1x1 ops have scalar operands
            # (no access-latency ack in their completion), so a tiny op right
            # after a chain op releases the consumer's count-based wait
            # before the producer's wide-write ack; same-engine FIFO write
            # commit keeps the data dependency safe.
            scr_act = consts.tile([1, 1], F32)
            nc.vector.memset(scr_act[:], 0.0)
            scr_dve = consts.tile([1, 1], F32)
            nc.vector.memset(scr_dve[:], 0.0)
            # Weight tiles; DMAs are emitted in the prologue interleaved with
            # chunk-0 input DMAs so the SP sequencer's ~0.6us-per-trigger
            # serialization doesn't push the first gate's g-pipeline out.
            wnm_hi_t = consts.tile([ZPK, RANK], BF16)
            wnm_lo_t = consts.tile([ZPK, RANK], BF16)
            bnm_t = consts.tile([RANK, 1], F32)
            v_t = consts.tile([DIM_HID, RANK], F32)
            wsr_t = consts.tile([KSR, DIM_HID], BF16)
            beye_t = consts.tile([DIM_HID, DIM_HID], F32)
            wout_t = consts.tile([DIM_HID, DIM_Y], F32)
            yscl_t = consts.tile([DIM_Y, NCHUNK], F32)

            # Bulk work is emitted in staggered phases (different step slots)
            # so each in-order engine reaches a bulk op only well after its
            # dependencies completed — a bulk op with unmet deps stalls the
            # engine and with it the recurrence chain.
            state = {"zt": {}, "gps": {}, "tg": {}, "g": {}, "rt": {},
                     "yps": {}, "xs": {}}

            def z_decode(szi, z_hi, z_lo, cs, uid):
                """Unpack nibble-packed z cols `cs`: b = 16h + l.
                z_hi <- h (even z dims), z_lo <- l (odd dims; the -8 of
                l's offset encoding is folded into the gate bias).  Two
                separate 8-row tiles: compute-engine accesses must start
                at a 32-aligned partition, so rows 8:16 of one tile are
                unreachable.  Float-only decode (Pool has no int8 bit or
                integer ops): h = round(b/16 - 0.46875) is exact via the
                rounding int8 cast (l/16 - 0.46875 lies in (-0.47, 0.47))."""
                zp = szi[DIM_S:SZROWS, cs]
                n = cs.stop - cs.start
                bf = zpk_pool.tile([ZPK, n], BF16, name=f"zbf_{uid}",
                                   tag=f"zbf{n}")
                nc.gpsimd.tensor_scalar_mul(bf[:], zp, 1.0)
                h8 = zpk_pool.tile([ZPK, n], I8, name=f"zh8_{uid}",
                                   tag=f"zh8{n}")
                nc.gpsimd.tensor_scalar(
                    h8[:], bf[:], 0.0625, -0.46875, op0=OP.mult, op1=OP.add
                )
                nc.gpsimd.tensor_scalar_mul(z_hi[:, cs], h8[:], 1.0)
                h16 = zpk_pool.tile([ZPK, n], BF16, name=f"zh16_{uid}",
                                    tag=f"zh16{n}")
                nc.gpsimd.tensor_scalar_mul(h16[:], h8[:], 16.0)
                nc.gpsimd.tensor_tensor(
                    z_lo[:, cs], bf[:], h16[:], op=OP.subtract
                )

            def bulk_dma(c):
                # int8 staging DMA + Pool-engine dequant casts to bf16.
                # Pool is off the chain, so a stall on the staging DMA is
                # harmless; the chain ops only wait the (gpsimd) casts.
                cb = c * NCOLS
                szi = szi_pool.tile([SZROWS, NCOLS], I8, name=f"szi_{c}",
                                    tag="szi")
                nc.sync.dma_start(szi[:], sz[:, cb : cb + NCOLS])
                z_hi = z_pool.tile([ZPK, NCOLS], BF16, name=f"z_hi_{c}",
                                   tag="z_hi")
                z_lo = z_pool.tile([ZPK, NCOLS], BF16, name=f"z_lo_{c}",
                                   tag="z_lo")
                rt = sr_pool.tile([KSR, NCOLS], BF16, name=f"rt_{c}", tag="rt")
                # zero the r + pad rows (pad rows pair with zero weights);
                # gpsimd so the DVE (gate engine) never stalls on it
                nc.gpsimd.memset(rt[0:32, :], 0.0)
                z_decode(szi, z_hi, z_lo, slice(0, NCOLS), f"c{c}")
                nc.gpsimd.tensor_scalar_mul(rt[32:64, :], szi[0:DIM_S, :], 1.0)
                # ones row (b_in); 64 is a 32-aligned partition offset
                nc.gpsimd.memset(rt[64:KSR, :], 1.0)
                state["zt"][c] = (z_hi, z_lo)
                state["rt"][c] = rt

            def _after(inst, anchor):
                if anchor is not None:
                    add_dep_helper(inst.ins, anchor.ins, sync=False,
                                   reason="bulk op ordered behind chain op")

            def bulk_mmg(c, piece, anchor=None):
                if piece == 0:
                    state["gps"][c] = g_psum.tile(
                        [RANK, NCOLS], F32, name=f"g_ps_{c}", tag="g_ps"
                    )
                p = piece * 128
                z_hi, z_lo = state["zt"][c]
                mm = nc.tensor.matmul(
                    state["gps"][c][:, p : p + 128], wnm_hi_t[:],
                    z_hi[:, p : p + 128], start=True, stop=False,
                )
                _after(mm, anchor)
                mm2 = nc.tensor.matmul(
                    state["gps"][c][:, p : p + 128], wnm_lo_t[:],
                    z_lo[:, p : p + 128], start=False, stop=True,
                )
                _after(mm2, anchor)

            def bulk_tg(c, piece, anchor=None):
                # sigmoid(w+b) = 0.5*tanh(0.5*w + 0.5*b) + 0.5 (stay on Tanh
                # table; b_nm rides the ACT bias operand); 128-col pieces so
                # no single ACT op can block a tanh long
                if piece == 0:
                    state["tg"][c] = tg_pool.tile(
                        [RANK, NCOLS], F32, name=f"tg_t_{c}", tag="tg_t"
                    )
                p = piece * 128
                a = nc.scalar.activation(
                    state["tg"][c][:, p : p + 128],
                    state["gps"][c][:, p : p + 128], AF.Tanh, scale=0.5,
                    bias=bnm_t[:, 0:1],
                )
                _after(a, anchor)

            def bulk_gaffine(c, piece=None):
                # on gpsimd: keeps the DVE free for the chain's gate op
                if piece is None or piece == 0:
                    state["g"][c] = g_pool.tile(
                        [RANK, NCOLS], F32, name=f"g_t_{c}", tag="g_t"
                    )
                sl = slice(0, NCOLS) if piece is None else slice(
                    piece * 128, piece * 128 + 128
                )
                nc.gpsimd.tensor_scalar(
                    state["g"][c][:, sl], state["tg"][c][:, sl],
                    0.5, 0.5, op0=OP.mult, op1=OP.add,
                )

            def y_mm(c, piece, anchor=None):
                if piece == 0:
                    state["yps"][c] = y_psum.tile(
                        [DIM_Y, NCOLS], F32, name=f"y_ps_{c}", tag="y_ps"
                    )
                p = piece * 128
                mm = nc.tensor.matmul(
                    state["yps"][c][:, p : p + 128], wout_t[:],
                    state["xs"][c][:, p : p + 128], start=True, stop=True,
                )
                _after(mm, anchor)

            def y_scale(c, anchor=None):
                # per-(row, chunk) scale: absmax/YQDIV, and its reciprocal;
                # reads straight from PSUM (b_out is added host-side)
                yps = state["yps"][c]
                amax = consts.tile([DIM_Y, 1], F32, name=f"amax_{c}")
                rd = nc.vector.tensor_reduce(
                    amax[:], yps[:], axis=mybir.AxisListType.X,
                    op=OP.max, apply_absolute_value=True,
                )
                _after(rd, anchor)
                sc = nc.vector.tensor_scalar_mul(
                    yscl_t[:, c : c + 1], amax[:], 1.0 / YQDIV
                )
                _after(sc, anchor)
                inv = consts.tile([DIM_Y, 1], F32, name=f"inv_{c}")
                rc = nc.vector.reciprocal(inv[:], yscl_t[:, c : c + 1])
                _after(rc, anchor)
                state[f"inv_{c}"] = inv

            def y_quant(c, anchor=None):
                # f32 -> int8 (round-to-nearest on the cast), per-row scale
                qt = q_pool.tile([DIM_Y, NCOLS], I8, name=f"q_t_{c}", tag="q_t")
                qq = nc.vector.tensor_scalar(
                    qt[:], state["yps"][c][:], state[f"inv_{c}"][:, 0:1], None,
                    op0=OP.mult,
                )
                _after(qq, anchor)
                state[f"qt_{c}"] = qt

            def y_out_emit(c):
                nc.sync.dma_start(
                    y_out[ts(c, NCOLS), :].transpose([1, 0]),
                    state[f"qt_{c}"][:],
                )
                del state["yps"][c], state["xs"][c]
                del state[f"qt_{c}"], state[f"inv_{c}"]

            x_prev_ps = None        # PSUM bank holding x_j (state)
            x_prev_sbuf = x_init    # SBUF copy of the previous state
            LAST = NCHUNK - 1

            # pipeline fill: bulk for the first two chunks
            # Interleaved startup triggers: the chunk-0 g-pipeline (wnm,
            # z piece 0) and the first chain steps (x_in, V, beye, s, wsr)
            # come first; later-needed weights ride the SWDGE queue.
            nc.sync.dma_start(wnm_hi_t[:], wnm[0:ZPK, :])
            nc.sync.dma_start(wnm_lo_t[:], wnm[ZPK : 2 * ZPK, :])
            nc.sync.dma_start(bnm_t[:], bnm[:])
            szi0 = szi_pool.tile([SZROWS, NCOLS], I8, name="szi_0", tag="szi")
            # land the first 4 steps' s+z early so the first gate's
            # g-pipeline isn't behind the full-chunk transfer
            nc.sync.dma_start(szi0[:, 0:128], sz[:, 0:128])
            nc.sync.dma_start(v_t[:], v_w[:])
            nc.sync.dma_start(beye_t[:], beye_const[:])
            nc.sync.dma_start(szi0[:, 128:NCOLS], sz[:, 128:NCOLS])
            z_hi0 = z_pool.tile([ZPK, NCOLS], BF16, name="z_hi_0", tag="z_hi")
            z_lo0 = z_pool.tile([ZPK, NCOLS], BF16, name="z_lo_0", tag="z_lo")
            state["zt"][0] = (z_hi0, z_lo0)
            rt0 = sr_pool.tile([KSR, NCOLS], BF16, name="rt_0", tag="rt")
            state["rt"][0] = rt0
            nc.gpsimd.memset(rt0[0:32, :], 0.0)
            z_decode(szi0, z_hi0, z_lo0, slice(0, 128), "p0a")
            z_decode(szi0, z_hi0, z_lo0, slice(128, NCOLS), "p0b")
            nc.gpsimd.tensor_scalar_mul(rt0[32:64, :], szi0[0:DIM_S, :], 1.0)
            nc.gpsimd.memset(rt0[64:KSR, :], 1.0)
            nc.gpsimd.dma_start(wsr_t[:], wsr[:])
            nc.gpsimd.dma_start(wout_t[:], wout[:])
            bulk_dma(1)
            # chunk 0 fully pipelined piece-by-piece so the first gate's g
            # columns are ready as early as possible
            for p in range(4):
                bulk_mmg(0, p)
                bulk_tg(0, p)
                bulk_gaffine(0, p)
            for p in range(4):
                bulk_mmg(1, p)
            for p in range(4):
                bulk_tg(1, p)
            bulk_gaffine(1)

            def emit_offchain(c, jj, pe_a, dve_a, act_a):
                """Bulk work for step slot jj of chunk c — ordered behind
                the same step's chain op on each in-order engine so it lands
                in the idle gap behind the chain, never in front of it."""
                if jj == 0:
                    if c + 2 < NCHUNK:
                        bulk_dma(c + 2)
                elif jj in (1, 2, 3, 4) and c > 0:
                    y_mm(c - 1, jj - 1, pe_a)
                if jj == 5 and c > 0:
                    y_scale(c - 1, dve_a)
                elif jj == 6 and c > 0:
                    y_quant(c - 1, dve_a)
                elif jj == 8 and c > 0:
                    y_out_emit(c - 1)
                if jj in (4, 5, 6, 7) and c + 2 < NCHUNK:
                    bulk_mmg(c + 2, jj - 4, pe_a)
                elif jj in (8, 10, 12, 14) and c + 2 < NCHUNK:
                    bulk_tg(c + 2, (jj - 8) // 2, act_a)
                elif jj == 15 and c + 2 < NCHUNK:
                    bulk_gaffine(c + 2)
                if jj == 13 and c + 1 < NCHUNK:
                    # Semaphore absorbers: make PE/DVE observe the next
                    # chunk's HWDGE/gpsimd producer semaphores via throwaway
                    # reads, so the chunk's first chain ops need only one
                    # wait (a 2-wait op gets split into an event-semaphore
                    # pair, adding ~100ns of sequencer latency to the chain).
                    nrt = state["rt"][c + 1]
                    scr_ps = q_psum.tile([1, 1], F32, tag="q",
                                         name=f"scr_ps_{c}")
                    for row in (0, 32, 64):
                        ab = nc.tensor.matmul(
                            scr_ps[:], nrt[row : row + 1, 0:1],
                            nrt[row : row + 1, 0:1], start=True, stop=True,
                        )
                        _after(ab, pe_a)
                    ab3 = nc.vector.tensor_tensor(
                        scr_sb[:], state["g"][c + 1][0:1, 0:1],
                        state["g"][c + 1][0:1, 0:1], op=OP.mult,
                    )
                    _after(ab3, dve_a)

            for j in range(TP):
                c = j // CH
                jj = j % CH
                if jj == 0:
                    state["xs"][c] = xs_pool.tile(
                        [DIM_HID, NCOLS], F32, name=f"xs_{c}", tag="xs"
                    )

                rt = state["rt"][c]
                g_t = state["g"][c]
                col = ts(jj, BL)

                # ---- critical chain: tanh -> mm1 -> gate -> mm_sr ----
                h_t = h_pool.tile([DIM_HID, BL], F32, name=f"h_{j}", tag="h")
                if x_prev_ps is None:
                    th = nc.scalar.activation(h_t[:], x_init[:], AF.Tanh)
                else:
                    th = nc.scalar.activation(h_t[:], x_prev_ps[:], AF.Tanh)
                # early semaphore flush for mm1 (see scr_act comment)
                fl_a = nc.scalar.activation(scr_act[:], scr_act[:], AF.Copy)
                _after(fl_a, th)

                # off-chain: evacuate x_j to SBUF (y-path + next mm_I rhs)
                if j > 0:
                    pc, pj = (j - 1) // CH, (j - 1) % CH
                    x_sb = state["xs"][pc][:, ts(pj, BL)]
                    cp = nc.scalar.activation(x_sb, x_prev_ps[:], AF.Copy)
                    _after(cp, fl_a)
                    # flush the copy too: mm_I waits it, and PE's in-order
                    # queue would otherwise hold mm_sr behind the copy's ack
                    fl_c = nc.scalar.activation(scr_act[:], scr_act[:], AF.Copy)
                    _after(fl_c, cp)
                    x_prev_sbuf = x_sb

                q_ps = q_psum.tile([RANK, BL], F32, name=f"q_{j}", tag="q")
                nc.tensor.matmul(q_ps[:], v_t[:], h_t[:], start=True, stop=True)

                gate = nc.vector.tensor_tensor(
                    rt[0:RANK, col], q_ps[:], g_t[:, col], op=OP.mult
                )
                # early semaphore flush for mm_sr (see scr_act comment)
                fl_d = nc.vector.tensor_scalar_mul(scr_dve[:], scr_dve[:], 1.0)
                _after(fl_d, gate)

                x_ps = x_psum.tile([DIM_HID, BL], F32, name=f"x_{j}", tag="x")
                # beta*x_{j-1} into the bank (off-chain: only needs the
                # SBUF copy of x_{j-1}), then accumulate the fused
                # alpha*(U r + b + W s) on top.
                nc.tensor.matmul(
                    x_ps[:], beye_t[:], x_prev_sbuf, start=True, stop=False
                )
                mm_sr = nc.tensor.matmul(
                    x_ps[:], wsr_t[:], rt[:, col], start=False, stop=True
                )
                x_prev_ps = x_ps

                emit_offchain(c, jj, mm_sr, gate, cp if j > 0 else th)

            # final state evacuation + last chunk's y + state handoff
            last = state["xs"][LAST][:, ts(CH - 1, BL)]
            nc.scalar.activation(last, x_prev_ps[:], AF.Copy)
            nc.sync.dma_start(x_out[:], last)
            for p in range(4):
                y_mm(LAST, p)
            y_scale(LAST)
            y_quant(LAST)
            y_out_emit(LAST)
            nc.sync.dma_start(
                y_out[PCOLS : PCOLS + 4 * NCHUNK, :].transpose([1, 0]),
                yscl_t[:].bitcast(I8),
            )

    nc.finalize()
    return nc


def _get_module(tp):
    key = f"nc_{tp}"
    if key not in _BUILT:
        _BUILT[key] = _build_module(tp)
    return _BUILT[key]


_WEIGHT_KEYS = ("U", "V", "W_in", "b_in", "W_out", "b_out", "W_nm", "b_nm")


def _prep_weights(inputs):
    U = np.asarray(inputs["U"], dtype=np.float32)
    V = np.asarray(inputs["V"], dtype=np.float32)
    W_in = np.asarray(inputs["W_in"], dtype=np.float32)
    b_in = np.asarray(inputs["b_in"], dtype=np.float32)
    W_out = np.asarray(inputs["W_out"], dtype=np.float32)
    b_out = np.asarray(inputs["b_out"], dtype=np.float32)
    W_nm = np.asarray(inputs["W_nm"], dtype=np.float32)
    b_nm = np.asarray(inputs["b_nm"], dtype=np.float32)

    # s arrives as raw int8 levels; the quant scale folds into the weights
    wsr = np.concatenate(
        [
            ALPHA * U.T,
            np.zeros((16, DIM_HID), dtype=np.float32),
            (ALPHA * QSCALE) * W_in.T,
            ALPHA * b_in[None, :],
        ],
        axis=0,
    ).astype(NP_BF16)

    def rep(a):
        return np.ascontiguousarray(
            np.broadcast_to(a[None], (NCORES, *a.shape))
        ).reshape(NCORES * a.shape[0], *a.shape[1:])

    # z arrives nibble-packed: device z rows are [even dims (h); odd dims
    # (l = z_odd_levels + 8)], so permute W_nm rows to match and fold the
    # constant -8 of l's encoding into the gate bias.
    perm = list(range(0, DIM_Z, 2)) + list(range(1, DIM_Z, 2))
    wnm_p = (QSCALE_Z * W_nm.T[perm]).astype(NP_BF16)
    bnm_eff = b_nm - 8.0 * QSCALE_Z * W_nm[:, 1::2].sum(axis=1)

    return {
        "v_w": rep(np.ascontiguousarray(V)),
        "wsr": rep(wsr),
        "wnm": rep(wnm_p),
        "bnm": rep((0.5 * bnm_eff).reshape(RANK, 1).astype(np.float32)),
        "wout": rep(np.ascontiguousarray(W_out.T)),
    }


def _get_weight_dev(inputs, sh, pool):
    """Device-resident weight arrays, cached across calls on content."""
    import jax

    fp = b"".join(
        np.ascontiguousarray(np.asarray(inputs[k])).tobytes()
        for k in _WEIGHT_KEYS
    )
    if _BUILT.get("wfp") == fp:
        return _BUILT["wdev"]

    wconcat = _prep_weights(inputs)

    def bput(a):
        x = jax.device_put(a, sh)
        x.block_until_ready()
        return x

    futs = {n: pool.submit(bput, a) for n, a in wconcat.items()}
    wdev = {n: f.result() for n, f in futs.items()}
    # initial state: zeros, uploaded once
    x0 = np.zeros((NCORES * DIM_HID, BL), np.float32)
    wdev["x0"] = pool.submit(bput, x0).result()
    _BUILT["wfp"] = fp
    _BUILT["wdev"] = wdev
    return wdev


def _quant_into(src, scale, qmax, tmp, i8buf):
    """clip(rint(src/scale), -qmax, qmax) -> int8, via preallocated bufs."""
    np.multiply(src, 1.0 / scale, out=tmp)
    np.rint(tmp, out=tmp)
    np.clip(tmp, -qmax, qmax, out=tmp)
    np.copyto(i8buf, tmp, casting="unsafe")
    return i8buf


TPS = 128                   # t-steps per prep sub-slice


def _prep_sub(s, z, p, i):
    """Quantize+transpose the i-th t-slice of piece p into its out buffer.

    Scratch buffers are shared (prep runs inline on the single cpu) and
    reused across calls (the previous call's uploads finished with them
    before the next call starts).
    """
    if "qtmp" not in _BUILT:
        _BUILT["qtmp"] = (
            np.empty((TPS, B, DIM_S), np.float32),
            np.empty((TPS, B, DIM_Z), np.float32),
            np.empty((TPS, B, DIM_S), np.int8),
            np.empty((TPS, B, DIM_Z), np.int8),
        )
        _BUILT["qout"] = [
            np.empty((NCORES, SZROWS, tp * BL), np.int8) for tp in PIECES
        ]
    stmp, ztmp, si8, zi8 = _BUILT["qtmp"]
    out = _BUILT["qout"][p]
    tp = PIECES[p]
    t0 = OFFS[p] + i * TPS
    sl = slice(t0, t0 + TPS)
    si = _quant_into(s[sl], QSCALE, 127, stmp, si8)
    zi = _quant_into(z[sl], QSCALE_Z, 7, ztmp, zi8)
    # nibble-pack: byte = (z_even << 4) | (z_odd + 8); view-cast keeps int8
    zu = zi.view(np.uint8)
    zpk = (
        (zu[..., 0::2] << 4) | ((zu[..., 1::2] + 8) & 15)
    ).view(np.int8)
    s_sl = si.reshape(TPS, NCORES, BL, DIM_S)
    z_sl = zpk.reshape(TPS, NCORES, BL, ZPK)
    tsl = slice(i * TPS, (i + 1) * TPS)
    for k in range(NCORES):
        out[k, 0:DIM_S].reshape(DIM_S, tp, BL)[:, tsl, :] = (
            s_sl[:, k].transpose(2, 0, 1)
        )
        out[k, DIM_S:SZROWS].reshape(ZPK, tp, BL)[:, tsl, :] = (
            z_sl[:, k].transpose(2, 0, 1)
        )


def _build_exec(tp):
    """Sharded jitted executable over the 8 cores for piece length tp."""
    import jax
    from jax.experimental.shard_map import shard_map
    from jax.sharding import Mesh, NamedSharding, PartitionSpec

    from concourse import bass2jax
    from concourse.bass2jax import _bass_exec_p, install_neuronx_cc_hook

    install_neuronx_cc_hook()
    nc = _get_module(tp)

    partition_name = (
        nc.partition_id_tensor.name if nc.partition_id_tensor else None
    )
    in_names, out_names, out_avals, zero_outs = [], [], [], []
    for alloc in nc.m.functions[0].allocations:
        if not isinstance(alloc, mybir.MemoryLocationSet):
            continue
        name = alloc.memorylocations[0].name
        if alloc.kind == "ExternalInput":
            if name != partition_name:
                in_names.append(name)
        elif alloc.kind == "ExternalOutput":
            shape = tuple(alloc.tensor_shape)
            dtype = mybir.dt.np(alloc.dtype)
            out_names.append(name)
            out_avals.append(jax.core.ShapedArray(shape, dtype))
            zero_outs.append(np.zeros(shape, dtype))
    n_params = len(in_names)
    in_names_all = list(in_names) + out_names
    if partition_name is not None:
        in_names_all.append(partition_name)

    def _body(*args):
        operands = list(args)
        if partition_name is not None:
            operands.append(bass2jax.partition_id_tensor())
        outs = _bass_exec_p.bind(
            *operands,
            out_avals=tuple(out_avals),
            in_names=tuple(in_names_all),
            out_names=tuple(out_names),
            lowering_input_output_aliases=(),
            sim_require_finite=True,
            sim_require_nnan=True,
            nc=nc,
        )
        return tuple(outs)

    devices = jax.devices()[:NCORES]
    mesh = Mesh(np.asarray(devices), ("core",))
    in_specs = (PartitionSpec("core"),) * (n_params + len(out_names))
    out_specs = (PartitionSpec("core"),) * len(out_names)
    # no donation: the kernel fully overwrites its outputs, so cached
    # on-device scratch buffers are reused as output operands every call
    sharded = jax.jit(
        shard_map(
            _body, mesh=mesh, in_specs=in_specs, out_specs=out_specs,
            check_rep=False,
        ),
        keep_unused=True,
    )
    sh = NamedSharding(mesh, PartitionSpec("core"))
    return sharded, sh, in_names, out_names, zero_outs


def _get_exec():
    """Per-piece executables + scratch.  Pieces of equal length share the
    compiled executable but each piece has its own output scratch set so
    piece p's download can overlap piece p+1's exec."""
    if "exec" in _BUILT:
        return _BUILT["exec"]

    import jax

    by_size = {tp: _build_exec(tp) for tp in sorted(set(PIECES))}
    sh = by_size[PIECES[0]][1]
    pieces = []
    for tp in PIECES:
        sharded, _, in_names, out_names, zero_outs = by_size[tp]
        scratch = [
            jax.device_put(
                np.zeros((NCORES * z0.shape[0], *z0.shape[1:]), z0.dtype), sh
            )
            for z0 in zero_outs
        ]
        pieces.append((sharded, in_names, out_names, scratch))
    _BUILT["exec"] = (pieces, sh)
    return _BUILT["exec"]


def run_sharded(inputs):
    """Run the SPMD kernel; returns the full [T, B, Y] output.

    Pipelined: piece p's sz upload rides the wire while piece p+1
    quantizes+transposes inline on the main thread; each piece's exec
    carries the recurrent state to the next on-device; piece p's y
    (int8 + embedded scales) downloads while later pieces upload and
    execute, and pool workers dequantize shards as they land.
    """
    import jax

    pieces, sh = _get_exec()
    pool = _get_pool()

    dev_w = _get_weight_dev(inputs, sh, pool)
    s = np.asarray(inputs["s"])
    z = np.asarray(inputs["z"])
    b_out = np.asarray(inputs["b_out"], dtype=np.float32).reshape(DIM_Y)

    # ping-pong full-output buffers: reuse pages across calls (the host
    # has ONE cpu; 64MB of first-touch page faults cost ~20ms) while the
    # previous call's returned array stays intact.
    if "ybufs" not in _BUILT:
        _BUILT["ybufs"] = [
            np.empty((T, B, DIM_Y), dtype=np.float32) for _ in range(2)
        ]
        _BUILT["yflip"] = 0
    _BUILT["yflip"] ^= 1
    y = _BUILT["ybufs"][_BUILT["yflip"]]

    def fetch(p, k, dq):
        tp = PIECES[p]
        pcols, nchunk = tp * BL, tp // CH
        qs = np.asarray(dq)       # [pcols + 4*nchunk, DIM_Y] int8
        # tail rows carry f32 scales: byte (r, 4c+b) lives at row 4c+b col r
        scl = np.ascontiguousarray(qs[pcols:].T).view(np.float32)
        scl2 = np.ascontiguousarray(scl.T)            # [nchunk, DIM_Y]
        # fused contiguous pass: int8 -> f32 upcast * per-(chunk, ydim) scale
        yf = np.multiply(
            qs[:pcols].reshape(nchunk, CH * BL, DIM_Y),
            scl2[:, None, :],
            dtype=np.float32,
        )
        # contiguous source; dest strided over t in 4KB row blocks
        np.add(
            yf.reshape(tp, BL, DIM_Y), b_out,
            out=y[OFFS[p] : OFFS[p] + tp, k * BL : (k + 1) * BL, :],
        )

    fetch_futs = []
    x_cur = dev_w["x0"]
    for p in range(NPIECE):
        sharded, in_names, out_names, scratch = pieces[p]
        # prep inline on the main thread (single cpu: pool fan-out just
        # thrashes; a small piece 0 gets the first upload out early)
        for i in range(PIECES[p] // TPS):
            _prep_sub(s, z, p, i)
        dev = dict(dev_w)
        dev["sz"] = jax.device_put(
            _BUILT["qout"][p].reshape(NCORES * SZROWS, PIECES[p] * BL), sh
        )
        dev["x_in"] = x_cur
        outs = sharded(*[dev[name] for name in in_names], *scratch)
        x_cur = outs[out_names.index("x_out")]
        yg = outs[out_names.index("y_out")]
        rows_per_core = PIECES[p] * BL + 4 * (PIECES[p] // CH)
        y_shards = sorted(
            ((sh_.index[0].start // rows_per_core, sh_.data)
             for sh_ in yg.addressable_shards), key=lambda t: t[0]
        )
        for _, d in y_shards:
            d.copy_to_host_async()
        for k, d in y_shards:
            fetch_futs.append(pool.submit(fetch, p, k, d))

    for f in fetch_futs:
        f.result()
    return y


def kernel(**inputs):
    return run_sharded(inputs)
